# revision 34
# baseline (speedup 1.0000x reference)
"""Causal self-attention (B=4, T=2048, E=1024, H=16, D=64) on 8 trn2 cores.

Sharding: core c -> (batch b = c//2, head-group g = c%2 of 8 heads).
Each core computes qkv projection + RoPE + causal attention + its partial
output projection for its (batch, head-group); host sums the partials.

Device data layout is feature-major ("T" suffix = [features, tokens]):
scores are computed k-major (S.T blocks [tk=128, tq]) so causal masking
skips ~half the matmuls, and softmax normalization comes from ones-columns
in the v operand of the PV matmul (the denominators land in PSUM partition
rows 64 / 63 at zero extra matmul cost).

Softmax normalize path (per 512-token q chunk): the two heads' denominator
rows sit on adjacent partitions (64 for head lo, 63 for head hi), get
fast-reciprocal'd into one SBUF tile, and a single K=2 ones-weight matmul
broadcasts both across all 128 partitions in one PSUM bank -- no DRAM
round-trip, no cross-partition DMA.

All matmuls run in float32r (full PE rate for N>=256; the one 128-wide
diagonal score/PV block per chunk is widened to 256 to stay on that rate).
The BIR verifier requires float32r matmul operands to be *produced* as
float32r, so every matmul-feeding tile is declared float32r; engine inputs
are read via .bitcast(float32) where needed (same bits).
"""
import sys

sys.path.insert(0, "/opt/trn_rl_repo")

from contextlib import ExitStack

import numpy as np

import concourse.bass as bass
import concourse.bacc as bacc
import concourse.tile as tile
from concourse import mybir
from concourse.bass_utils import run_bass_kernel_spmd

B, T, E, H, D = 4, 2048, 1024, 16, 64
NCORES = 8
HG = H // 2          # heads per shard (8)
F = HG * D           # features per shard (512)
NPAIR = F // 128     # head pairs per shard (4)
NGRP = NPAIR // 2    # pair groups (2)
KE = E // 128        # contraction tiles over E (8)
NKT = T // 128       # k tiles (16)
TCH = 512            # t-chunk for qkv streaming
NCH = T // TCH       # chunks per group (4)
F32 = mybir.dt.float32
F32R = mybir.dt.float32r
EXP = mybir.ActivationFunctionType.Exp
LN = mybir.ActivationFunctionType.Ln

_NC_CACHE = {}


def _build_program():
    key = "nc"
    if key in _NC_CACHE:
        return _NC_CACHE[key]
    nc = bacc.Bacc("TRN2", target_bir_lowering=False, debug=False,
                   num_devices=NCORES)
    mm = nc.tensor.matmul
    xT = nc.dram_tensor("xT", [E, T], F32R, kind="ExternalInput").ap()
    wqT = nc.dram_tensor("wqT", [E, F], F32R, kind="ExternalInput").ap()
    wkT = nc.dram_tensor("wkT", [E, F], F32R, kind="ExternalInput").ap()
    wvT = nc.dram_tensor("wvT", [E, F], F32R, kind="ExternalInput").ap()
    wpT = nc.dram_tensor("wpT", [F, E], F32R, kind="ExternalInput").ap()
    cos2 = nc.dram_tensor("cos2", [128, T], F32, kind="ExternalInput").ap()
    ssp2 = nc.dram_tensor("ssp2", [128, T], F32, kind="ExternalInput").ap()
    perm = nc.dram_tensor("perm", [128, 128], F32R, kind="ExternalInput").ap()
    maskd = nc.dram_tensor("maskd", [128, 128], F32, kind="ExternalInput").ap()
    maskw = nc.dram_tensor("maskw", [128, 256], F32, kind="ExternalInput").ap()
    one128 = nc.dram_tensor("one128", [128, 128], F32R,
                            kind="ExternalInput").ap()
    ones16 = nc.dram_tensor("ones16", [128, NKT], F32R,
                            kind="ExternalInput").ap()
    ident = nc.dram_tensor("ident", [128, 128], F32R,
                           kind="ExternalInput").ap()
    outs = [nc.dram_tensor(f"outT{g}", [E, T], F32, kind="ExternalOutput").ap()
            for g in range(NGRP)]

    xT_r = xT.rearrange("(ke p) t -> p ke t", p=128)
    wq_r = wqT.rearrange("(ke p) j -> p ke j", p=128)
    wk_r = wkT.rearrange("(ke p) j -> p ke j", p=128)
    wv_r = wvT.rearrange("(ke p) j -> p ke j", p=128)
    wp_r = wpT.rearrange("(kf p) o -> p kf o", p=128)

    with tile.TileContext(nc) as tc:
        with ExitStack() as ctx:
            const = ctx.enter_context(tc.tile_pool(name="const", bufs=1))
            wgp = ctx.enter_context(tc.tile_pool(name="wgp", bufs=1))
            xp = ctx.enter_context(tc.tile_pool(name="xp", bufs=4))
            qkp = ctx.enter_context(tc.tile_pool(name="qkp", bufs=2))
            vp = ctx.enter_context(tc.tile_pool(name="vp", bufs=2))
            yp = ctx.enter_context(tc.tile_pool(name="yp", bufs=2))
            pp = ctx.enter_context(tc.tile_pool(name="pp", bufs=3))
            tmp = ctx.enter_context(tc.tile_pool(name="tmp", bufs=4))
            bcp = ctx.enter_context(tc.tile_pool(name="bcp", bufs=2))
            zzp = ctx.enter_context(tc.tile_pool(name="zzp", bufs=2))
            outp = ctx.enter_context(tc.tile_pool(name="outp", bufs=2))
            # PSUM: psS 2x[128,1024] = 4 banks, psA 2x[128,512] = 2,
            # psY 2x[128,512] = 2 -> 8 banks total.
            psA = ctx.enter_context(
                tc.tile_pool(name="psA", bufs=2, space="PSUM"))
            psS = ctx.enter_context(
                tc.tile_pool(name="psS", bufs=2, space="PSUM"))
            psY = ctx.enter_context(
                tc.tile_pool(name="psY", bufs=2, space="PSUM"))

            # group-0 weights + first x chunks first: they gate the first
            # matmul. Remaining consts overlap with compute.
            wg_tiles = [None, None]
            wg_tiles[0] = wgp.tile([128, KE, 768], F32R, tag="wg", name="wg0")
            j0 = 0
            nc.sync.dma_start(out=wg_tiles[0][:, :, 0:256],
                              in_=wq_r[:, :, j0:j0 + 256])
            nc.sync.dma_start(out=wg_tiles[0][:, :, 256:512],
                              in_=wk_r[:, :, j0:j0 + 256])
            nc.sync.dma_start(out=wg_tiles[0][:, :, 512:768],
                              in_=wv_r[:, :, j0:j0 + 256])

            xtiles = {}

            def issue_xc(ci):
                xca = xp.tile([128, KE // 2, TCH], F32R, tag="xc")
                xcb = xp.tile([128, KE // 2, TCH], F32R, tag="xc")
                tcs = (ci % NCH) * TCH
                nc.sync.dma_start(out=xca,
                                  in_=xT_r[:, 0:KE // 2, tcs:tcs + TCH])
                nc.sync.dma_start(out=xcb,
                                  in_=xT_r[:, KE // 2:KE, tcs:tcs + TCH])
                xtiles[ci] = (xca, xcb)

            issue_xc(0)
            issue_xc(1)

            c_cos = const.tile([128, T], F32, tag="cos")
            c_ssp = const.tile([128, T], F32, tag="ssp")
            c_perm = const.tile([128, 128], F32R, tag="perm")
            c_mask = const.tile([128, 128], F32, tag="mask")
            c_maskw = const.tile([128, 256], F32, tag="maskw")
            c_one = const.tile([128, 128], F32R, tag="one")
            c_ident = const.tile([128, 128], F32R, tag="ident")
            c_wp = const.tile([128, NPAIR, E], F32R, tag="wp")
            # only cos/ssp/perm gate early phase-A work; the rest are
            # emitted after phase A g0 so x-chunk DMAs aren't queued
            # behind them (they're not needed until phase B).
            nc.sync.dma_start(out=c_cos, in_=cos2)
            nc.sync.dma_start(out=c_ssp, in_=ssp2)
            nc.sync.dma_start(out=c_perm, in_=perm)

            pending = [None]

            def flush_pending():
                if pending[0] is not None:
                    pending[0]()
                    pending[0] = None

            for g in range(NGRP):
                # deferred drain from the previous group overlaps with this
                # group's first qkv matmuls
                flush_pending()
                # ---- phase A: qkv projection + rope for pairs 2g, 2g+1 ----
                wg = wg_tiles[g]

                pair_qk = []
                pair_v = []
                for pi in range(2):
                    qT = qkp.tile([128, T], F32R, tag="qT")
                    kT = qkp.tile([128, T], F32R, tag="kT")
                    # v3 cols per k tile: [v_lo(0:64) | 1 | v_hi(65:129) | 1]
                    # head-lo weights cols 0:65, head-hi cols 65:130; both
                    # produce psum rows 0:65 (y at 0:64, Z at 64).
                    v3 = vp.tile([128, NKT, 130], F32R, tag="v3")
                    nc.sync.dma_start(out=v3[:, :, 64], in_=ones16)
                    nc.sync.dma_start(out=v3[:, :, 129], in_=ones16)
                    pair_qk.append((qT, kT))
                    pair_v.append(v3)

                for ci in range(NCH):
                    tcs = ci * TCH
                    xca, xcb = xtiles[g * NCH + ci]

                    def xk(ke):
                        return (xca if ke < KE // 2
                                else xcb)[:, ke % (KE // 2), :]
                    tcol = slice(tcs, tcs + TCH)
                    # all q/k matmuls of the chunk first (PE stays busy),
                    # then the rope vector work + perm matmuls, then v.
                    pst = []
                    for pi in range(2):
                        ps = psS.tile([128, 1024], F32, tag="psS")
                        for si in range(2):          # 0 = q, 1 = k
                            wcol = 256 * si + 128 * pi
                            for ke in range(KE):
                                mm(ps[:, 512 * si:512 * si + 512],
                                   wg[:, ke, wcol:wcol + 128],
                                   xk(ke), start=(ke == 0),
                                   stop=(ke == KE - 1), skip_group_check=True)
                        pst.append(ps)
                    # rope muls on vector: bt = ps*ssp, dst = ps*cos
                    bts = []
                    for pi in range(2):
                        for si in range(2):
                            dst = pair_qk[pi][si]
                            src = pst[pi][:, 512 * si:512 * si + 512]
                            nc.vector.tensor_mul(dst[:, tcol], src,
                                                 c_cos[:, tcol])
                            bt = tmp.tile([128, TCH], F32R, tag="bt")
                            nc.vector.tensor_mul(bt, src, c_ssp[:, tcol])
                            bts.append(bt)
                    # v for both pairs (N=256 across the group's 256 cols);
                    # emitted before the perm matmuls so the PE isn't
                    # blocked behind them waiting on the vector muls.
                    for ti in range(TCH // 128):
                        tt = (tcs + ti * 128) // 128
                        psv = psY.tile([128, 256], F32, tag="psY")
                        for ke in range(KE):
                            mm(psv, xk(ke)[:, 128 * ti:128 * ti + 128],
                               wg[:, ke, 512:768], start=(ke == 0),
                               stop=(ke == KE - 1), skip_group_check=True)
                        for pi in range(2):
                            nc.scalar.copy(
                                pair_v[pi][:, tt, 0:64],
                                psv[:, 128 * pi:128 * pi + 64])
                            nc.scalar.copy(
                                pair_v[pi][:, tt, 65:129],
                                psv[:, 128 * pi + 64:128 * pi + 128])
                    # rope rotate term: psw = perm @ bt, dst += psw
                    psws = []
                    for bi in range(4):
                        psw = psA.tile([128, TCH], F32, tag="psA")
                        mm(psw, c_perm, bts[bi], start=True, stop=True,
                           skip_group_check=True)
                        psws.append(psw)
                    for pi in range(2):
                        for si in range(2):
                            dst = pair_qk[pi][si]
                            nc.vector.tensor_add(
                                dst[:, tcol], dst[:, tcol].bitcast(F32),
                                psws[2 * pi + si])
                    # prefetch 2 chunks ahead (emitted after this chunk's
                    # reads so the buffer-reuse DMA waits for them)
                    if ci + 2 < NCH:
                        issue_xc(g * NCH + ci + 2)

                if g == 0:
                    nc.sync.dma_start(out=c_mask, in_=maskd)
                    nc.sync.dma_start(out=c_maskw, in_=maskw)
                    nc.sync.dma_start(out=c_one, in_=one128)
                    nc.sync.dma_start(out=c_ident, in_=ident)
                    nc.sync.dma_start(out=c_wp, in_=wp_r)
                if g + 1 < NGRP:
                    # prefetch next group's weights + first x chunks while
                    # this group's attention runs.
                    wg_tiles[g + 1] = wgp.tile([128, KE, 768], F32R, tag="wg",
                                               name="wg1")
                    j0 = 256 * (g + 1)
                    nc.sync.dma_start(out=wg_tiles[g + 1][:, :, 0:256],
                                      in_=wq_r[:, :, j0:j0 + 256])
                    nc.sync.dma_start(out=wg_tiles[g + 1][:, :, 256:512],
                                      in_=wk_r[:, :, j0:j0 + 256])
                    nc.sync.dma_start(out=wg_tiles[g + 1][:, :, 512:768],
                                      in_=wv_r[:, :, j0:j0 + 256])
                    issue_xc((g + 1) * NCH)
                    issue_xc((g + 1) * NCH + 1)

                # ---- phase B: attention, pairs in sequence ----
                # The per-chunk softmax drain is emitted *deferred*: it is
                # flushed in the middle of the next chunk's score matmuls so
                # its PE ops (transposes) never block the PE on the serial
                # vector/gpsimd stages, and PV runs two k-tiles behind the
                # scores so exp+mask latency is fully hidden.
                y_tiles = [None, None]

                def make_drain(psy0, psy1, yT, qb, lp, g=g,
                               y_tiles=y_tiles):
                    def emit():
                        # normalize via transpose-reciprocal: stage both Z
                        # rows to SBUF, PE-transpose so the 1024 values sit
                        # 4-per-partition, run the exact DVE reciprocal
                        # there cheaply, replicate per-partition on gpsimd,
                        # and PE-transpose back into broadcast form.
                        zz = zzp.tile([128, 1024], F32R, tag="zz", bufs=1,
                                      name="zz")
                        nc.vector.tensor_copy(zz[64:65, 0:512],
                                              psy0[64:65, :])
                        nc.vector.tensor_copy(zz[64:65, 512:1024],
                                              psy1[64:65, :])
                        tp0 = psA.tile([128, 512], F32R, tag="psA",
                                       name="tp0")
                        tp1 = psA.tile([128, 512], F32R, tag="psA",
                                       name="tp1")
                        for b in range(4):
                            nc.tensor.transpose(
                                tp0[:, 128 * b:128 * b + 128],
                                zz[:, 128 * b:128 * b + 128], c_ident)
                            nc.tensor.transpose(
                                tp1[:, 128 * b:128 * b + 128],
                                zz[:, 512 + 128 * b:512 + 128 * b + 128],
                                c_ident)
                        rcol = bcp.tile([128, 8], F32, tag="rcol",
                                        name="rcol")
                        tp0v = tp0.rearrange("p (b c) -> p b c", c=128)
                        tp1v = tp1.rearrange("p (b c) -> p b c", c=128)
                        nc.vector.reciprocal(rcol[:, 0:4],
                                             tp0v[:, :, 64].bitcast(F32))
                        nc.vector.reciprocal(rcol[:, 4:8],
                                             tp1v[:, :, 64].bitcast(F32))
                        rep = tmp.tile([128, 8, 64], F32R, tag="rep",
                                       bufs=2, name="rep")
                        for hb in range(8):
                            nc.gpsimd.tensor_scalar_mul(
                                rep[:, hb, :],
                                c_one[:, 0:64].bitcast(F32),
                                rcol[:, hb:hb + 1])
                        bc0 = psA.tile([128, 512], F32R, tag="psA",
                                       name="bc0")
                        bc1 = psA.tile([128, 512], F32R, tag="psA",
                                       name="bc1")
                        for b in range(4):
                            nc.tensor.transpose(
                                bc0[0:64, 128 * b:128 * b + 128],
                                rep[:, b, :], c_ident)
                            nc.tensor.transpose(
                                bc1[0:64, 128 * b:128 * b + 128],
                                rep[:, 4 + b, :], c_ident)
                        bcs0 = bcp.tile([128, 512], F32, tag="bc",
                                        name="bcs0")
                        nc.vector.tensor_copy(bcs0[0:64, :],
                                              bc0[0:64, :].bitcast(F32))
                        bcs1 = bcp.tile([128, 512], F32, tag="bc",
                                        name="bcs1")
                        nc.vector.tensor_copy(bcs1[0:64, :],
                                              bc1[0:64, :].bitcast(F32))
                        nc.vector.tensor_mul(yT[0:64, qb:qb + 512],
                                             psy0[0:64, :], bcs0[0:64, :])
                        ym = tmp.tile([128, 512], F32R, tag="ym", bufs=2,
                                      name="ym")
                        nc.vector.tensor_mul(ym[0:64, :],
                                             psy1[0:64, :], bcs1[0:64, :])
                        nc.sync.dma_start(out=yT[64:128, qb:qb + 512],
                                          in_=ym[0:64, :])
                        if lp == 1:
                            # partial out-projection for this group's two
                            # pairs over this 512-col chunk
                            for mo in range(E // 128):
                                po = psA.tile([128, 512], F32, tag="psA",
                                              name="po")
                                for kp in range(2):
                                    mm(po,
                                       c_wp[:, 2 * g + kp,
                                            128 * mo:128 * mo + 128],
                                       y_tiles[kp][:, qb:qb + 512],
                                       start=(kp == 0), stop=(kp == 1),
                                       skip_group_check=True)
                                ost = outp.tile([128, 512], F32, tag="ost",
                                                name="ost")
                                nc.scalar.copy(ost, po)
                                nc.sync.dma_start(
                                    out=outs[g][128 * mo:128 * mo + 128,
                                                qb:qb + 512],
                                    in_=ost)
                    return emit

                for lp in range(2):
                    qT, kT = pair_qk[lp]
                    v3 = pair_v[lp]
                    yT = yp.tile([128, T], F32R, tag="yT")
                    y_tiles[lp] = yT
                    for qq in range(4):  # 512-col q chunks
                        qb = 512 * qq
                        kts = list(range(4 * qq + 4))
                        last = kts[-1]
                        psy0 = psy1 = None
                        hist = []

                        def emit_pv(pkt, ppt, pcol, stop):
                            mm(psy0[0:65, pcol:512], v3[:, pkt, 0:65],
                               ppt[:, pcol:512], start=(pkt == 0),
                               stop=stop, skip_group_check=True)
                            mm(psy1[0:65, pcol:512], v3[:, pkt, 65:130],
                               ppt[:, 512 + pcol:1024], start=(pkt == 0),
                               stop=stop, skip_group_check=True)

                        for i, kt in enumerate(kts):
                            diag = 128 * kt - qb       # >=0 on diag blocks
                            col_lo = max(0, min(diag, 256))
                            pS = psS.tile([128, 1024], F32, tag="psS",
                                          name="pS")
                            for hl in range(2):
                                hr = 64 * hl
                                mm(pS[:, 512 * hl + col_lo:512 * hl + 512],
                                   kT[hr:hr + 64, 128 * kt:128 * kt + 128],
                                   qT[hr:hr + 64, qb + col_lo:qb + 512],
                                   start=True, stop=True,
                                   skip_group_check=True)
                            pt = pp.tile([128, 1024], F32R, tag="pt",
                                         name="pt")
                            pS2 = pS.rearrange("p (h c) -> p h c", h=2)
                            pt2 = pt.rearrange("p (h c) -> p h c", h=2)
                            nc.scalar.activation(
                                pt2[:, :, col_lo:512],
                                pS2[:, :, col_lo:512], EXP, scale=0.125)
                            if diag >= 0:  # diagonal block: mask both heads
                                if diag == 384:  # widened: zero 256:384 too
                                    for hl in range(2):
                                        o = 512 * hl + 256
                                        nc.gpsimd.tensor_mul(
                                            pt[:, o:o + 256],
                                            pt[:, o:o + 256].bitcast(F32),
                                            c_maskw)
                                else:
                                    for hl in range(2):
                                        o = 512 * hl + diag
                                        nc.gpsimd.tensor_mul(
                                            pt[:, o:o + 128],
                                            pt[:, o:o + 128].bitcast(F32),
                                            c_mask)
                            hist.append((kt, pt, col_lo))
                            if i == 2:
                                # previous chunk's drain goes here, behind
                                # this chunk's first six score matmuls
                                flush_pending()
                                psy0 = psY.tile([128, 512], F32, tag="psY",
                                                name="psy0")
                                psy1 = psY.tile([128, 512], F32, tag="psY",
                                                name="psy1")
                            if i >= 2:
                                emit_pv(*hist[i - 2], stop=False)
                        emit_pv(*hist[-2], stop=False)
                        emit_pv(*hist[-1], stop=True)
                        pending[0] = make_drain(psy0, psy1, yT, qb, lp)

            flush_pending()

    nc.compile()
    _NC_CACHE[key] = nc
    return nc


def _host_tables():
    inv_freq = 1.0 / (10000.0 ** (np.arange(0, D, 2, dtype=np.float32) / D))
    t = np.arange(T, dtype=np.float32)
    freqs = np.outer(t, inv_freq)                     # [T, 32]
    emb = np.concatenate([freqs, freqs], -1)          # [T, 64]
    cos_t = np.cos(emb).T.astype(np.float32)          # [64, T]
    sin_t = np.sin(emb).T.astype(np.float32)
    # rope(x)[d] = x[d]*cos[d] + x[d^1]*ssin[d],
    #   ssin[2i] = -sin[2i], ssin[2i+1] = +sin[2i+1]
    # device computes perm @ (x * ssp), so ssp[d] = ssin[d^1]:
    ssp = np.empty_like(sin_t)
    ssp[0::2] = sin_t[1::2]       # even d: +sin(emb[d+1])
    ssp[1::2] = -sin_t[0::2]      # odd d:  -sin(emb[d-1])
    cos2 = np.concatenate([cos_t, cos_t], 0)          # [128, T]
    ssp2 = np.concatenate([ssp, ssp], 0)
    d = np.arange(128)
    perm = (d[None, :] == (d ^ 1)[:, None]).astype(np.float32)
    r = np.arange(128)
    maskd = (r[:, None] <= r[None, :]).astype(np.float32)
    maskw = np.concatenate(
        [np.zeros((128, 128), np.float32), maskd], axis=1)
    return cos2, ssp2, perm, maskd, maskw


def kernel(x, w_attn, w_proj):
    x = np.asarray(x, dtype=np.float32)
    w_attn = np.asarray(w_attn, dtype=np.float32)
    w_proj = np.asarray(w_proj, dtype=np.float32)
    cos2, ssp2, perm, maskd, maskw = _host_tables()
    one128 = np.ones((128, 128), dtype=np.float32)
    ones16 = np.ones((128, NKT), dtype=np.float32)
    ident = np.eye(128, dtype=np.float32)

    nc = _build_program()
    in_maps = []
    for c in range(NCORES):
        b, g = c // 2, c % 2
        j0 = g * F
        in_maps.append({
            "xT": np.ascontiguousarray(x[b].T),
            "wqT": np.ascontiguousarray(w_attn[j0:j0 + F].T),
            "wkT": np.ascontiguousarray(w_attn[E + j0:E + j0 + F].T),
            "wvT": np.ascontiguousarray(w_attn[2 * E + j0:2 * E + j0 + F].T),
            "wpT": np.ascontiguousarray(w_proj[:, j0:j0 + F].T),
            "cos2": cos2, "ssp2": ssp2, "perm": perm, "maskd": maskd,
            "maskw": maskw, "one128": one128, "ones16": ones16,
            "ident": ident,
        })
    res = run_bass_kernel_spmd(nc, in_maps, core_ids=list(range(NCORES)))
    out = np.empty((B, T, E), dtype=np.float32)
    for b in range(B):
        acc = (res.results[2 * b]["outT0"] + res.results[2 * b]["outT1"] +
               res.results[2 * b + 1]["outT0"] +
               res.results[2 * b + 1]["outT1"])
        out[b] = acc.T
    return out
